# revision 5
# baseline (speedup 1.0000x reference)
"""Trainium2 Bass kernel for mixed-head attention (CIM attention).

Reference computation (per batch element b):
    qkv  = x @ w_qkv.T                                  [N, 3C]
    q,k,v split into H=4 heads of HD=128
    S_h  = (q_h @ k_h.T) * SCALE                        [N, N] per head
    S'_i = sum_h M[i,h] * S_h        (CIM head mix)
    A_i  = softmax(S'_i, axis=-1)
    O_i  = A_i @ v_i
    out  = concat_i(O_i) @ w_proj.T + b_proj

Distribution: data-parallel over B=8, one batch element per NeuronCore.
No collectives needed; host shards/gathers.

Single-core algorithm (all matmuls bf16 with fp32 PSUM accumulation):
  - Host ships x^T, w_qkv^T, w_proj^T pre-transposed, pre-cast to bf16 and
    pre-packed into flat "SBUF image" layouts; inputs stream in first-use
    order as 128 KB pieces split across the three DMA queues; gpsimd runs
    the memsets first so the ones tile is ready for a short burst of
    throwaway ones*zeros matmuls that warms the PE p-state while the first
    operands are in flight.
  - The QK projection is split per 512-column n/m chunk and emitted
    K(ch0)+Q(ch0) first, so the ch0 score phase starts after only half the
    projection work (~1 MB of input instead of 2 MB); K(ch1), Q(ch1) and V
    are emitted between ch0 score groups and hide under them.
  - The CIM mix is folded into Q: Qhat_i[(h,d), n] = M[i,h]*SCALE*Q_h[d, n].
    Each Q projection half is cast once PSUM->SBUF (ACT), then scaled into
    the 4 i-variants on DVE (per-partition scalar).  The score matmul
    contracts over all 512 (h,d) pairs:
    S'_i^T[m, n] = sum_{(h,d)} K[(h,d), m] * Qhat_i[(h,d), n].
  - Scores live in [m_part, n_free] ("S^T") layout so exp is elementwise and
    attn@v needs no transpose: O_i^T[d, n] = sum_m V[m, d] * expS_i^T[m, n].
  - Score tiles are computed in [128, 1024] 2-bank PSUM pairs (two m-blocks
    side by side), with an i-pair inner loop so each K stationary load is
    reused by 4 consecutive matmuls; one wide exp per pair halves the ACT
    instruction count (and the semaphore count, which sets the length of
    the framework's teardown ladder).
  - Softmax normalization is deferred past attn@v.  Denominators come from
    a DVE add chain over the wide exp tiles feeding one ones[128,128]
    stationary matmul whose M=128 output broadcasts the rowsum to all
    partitions; 1/rowsum via reciprocal_approx_fast, applied to O^T with
    tensor_mul.  Per chunk the tail runs attnv before each head's rowsum
    so the DVE chain is off the PE critical path, and every tail is
    emitted after one score group of the following phase so the exp
    latency of its last tile is hidden.
  - The last chunk runs its score groups head-pair-outer: the pair-0 tail
    (attnv + normalize for heads 0,1) hides under pair-1's score groups,
    so only one head pair's normalize chain sits on the critical path at
    the very end.  Its proj is emitted ib0-2 first for both PSUM halves
    and ib3 last, so only the final two matmuls wait on head 3.
  - proj runs per 512-column chunk as soon as that chunk's four heads are
    normalized (chunk 0's proj is emitted after the first score group of
    chunk 1), with ib-ascending accumulation; half the output (1 MB) is
    written to HBM long before the kernel ends and the output DMAs
    alternate between the two HWDGE queues (the tail quarter rides the
    idle gpsimd queue so sync/scalar each issue only one final trigger).
"""

import os
import sys

for _p in ("/opt/trn_rl_repo",):
    if os.path.isdir(_p) and _p not in sys.path:
        sys.path.insert(0, _p)

import numpy as np
import ml_dtypes

import concourse.bass as bass
import concourse.tile as tile
from concourse import bacc, mybir
from concourse.bass_utils import run_bass_kernel_spmd

B, N, C, H = 8, 1024, 512, 4
HD = C // H          # 128
SCALE = HD ** -0.5
NCORES = 8
P = 128              # partitions
NCH = N // 512       # 512-wide free-dim chunks per N
NB = N // P          # 128-row blocks per N
CB = C // P          # 128-row blocks per C

BF16 = mybir.dt.bfloat16
FP32 = mybir.dt.float32
AF = mybir.ActivationFunctionType

N_WARMUP_MM = 8      # ones*zeros p-state warmers while first DMAs land


def _mix_matrix_np(w_main: np.ndarray, w_rest: np.ndarray) -> np.ndarray:
    rows = np.repeat(np.arange(H), H - 1)
    cols = np.array([[j for j in range(H) if j != i] for i in range(H)]).ravel()
    M = np.zeros((H, H), dtype=np.float64)
    M[rows, cols] = w_rest.astype(np.float64).ravel()
    M += np.diag(w_main.astype(np.float64))
    return M


def build_graph():
    nc = bacc.Bacc(
        "TRN2",
        target_bir_lowering=False,
        debug=False,
        num_devices=NCORES,
    )

    # xT: ch-major [128, NCH*(CB*512)]; block (ch, cb) at ch*2048 + cb*512
    xT = nc.dram_tensor("xT", [P, NCH * CB * 512], BF16, kind="ExternalInput").ap()
    # wqk: jb-major [128, 8*(CB*128)]; block (jb, cb) at jb*512 + cb*128
    wqk = nc.dram_tensor("wqk", [P, 2 * H * CB * P], BF16, kind="ExternalInput").ap()
    # wv / wpT: cb-major [128, CB*512]
    wv = nc.dram_tensor("wv", [P, CB * C], BF16, kind="ExternalInput").ap()
    wpTp = nc.dram_tensor("wpTp", [P, CB * C], BF16, kind="ExternalInput").ap()
    bprow = nc.dram_tensor("bprow", [P, C], BF16, kind="ExternalInput").ap()
    qscales = nc.dram_tensor("qscales", [P, H * H], FP32, kind="ExternalInput").ap()
    # bf16 output (upcast host-side): halves the output-DMA tail; the
    # ~0.1% quantization is well inside the error budget
    out = nc.dram_tensor("out", [N, C], BF16, kind="ExternalOutput").ap()

    with tile.TileContext(nc, pool_alloc_mode="queue") as tc:
        with (
            tc.tile_pool(name="const", bufs=1) as cpool,
            tc.tile_pool(name="wts", bufs=1) as wpool,
            tc.tile_pool(name="qkv", bufs=1) as qkvpool,
            tc.tile_pool(name="es", bufs=16) as espool,
            tc.tile_pool(name="esum", bufs=8) as sumpool,
            tc.tile_pool(name="onorm", bufs=1) as opool,
            tc.tile_pool(name="outsb", bufs=3) as outpool,
            tc.tile_pool(name="ps", bufs=4, space="PSUM") as pspool,
        ):
            # gpsimd: memsets first, so ones_m is ready early for warmup
            ones_m = cpool.tile([P, P], BF16, tag="ones_m")
            nc.gpsimd.memset(ones_m[:], 1.0)
            warm = cpool.tile([P, 512], BF16, tag="warm")
            nc.gpsimd.memset(warm[:], 0.0)

            # ---- staged input DMA, three queues, first-use order ----
            wqkp = wpool.tile([P, 2 * H * CB * P], BF16, tag="wqkp", name="wqkp")
            xtp = wpool.tile([P, NCH * CB * 512], BF16, tag="xtp", name="xtp")
            wvp = wpool.tile([P, CB * C], BF16, tag="wvp", name="wvp")
            wpp = wpool.tile([P, CB * C], BF16, tag="wpp", name="wpp")
            qsc = cpool.tile([P, H * H], FP32, tag="qsc")
            bpr = cpool.tile([P, C], BF16, tag="bpr")

            def xq(p):
                return slice(p * 1024, (p + 1) * 1024)

            def hq(p):
                return slice(p * 512, (p + 1) * 512)

            # First-use order per queue, balanced so the prologue's needs
            # (x ch0 + all 8 w blocks, 1.5 MB) land in ~4.5 us: the K(ch0)
            # half-groups gate on {x_c0, j4, j5}, then {j0, j1}, {j6, j7},
            # {j2, j3}.  Late tensors (wv, wpT, bias) trail.
            nc.scalar.dma_start(qsc[:], qscales[:, :])
            nc.sync.dma_start(xtp[:, hq(0)], xT[:, hq(0)])        # c0 cb0
            nc.scalar.dma_start(xtp[:, hq(1)], xT[:, hq(1)])      # c0 cb1
            nc.gpsimd.dma_start(xtp[:, hq(2)], xT[:, hq(2)])      # c0 cb2
            nc.sync.dma_start(wqkp[:, hq(4)], wqk[:, hq(4)])      # j4 (K0)
            nc.scalar.dma_start(wqkp[:, hq(5)], wqk[:, hq(5)])    # j5 (K1)
            nc.gpsimd.dma_start(xtp[:, hq(3)], xT[:, hq(3)])      # c0 cb3
            nc.sync.dma_start(wqkp[:, hq(0)], wqk[:, hq(0)])      # j0 (Q0)
            nc.scalar.dma_start(wqkp[:, hq(1)], wqk[:, hq(1)])    # j1 (Q1)
            nc.gpsimd.dma_start(wqkp[:, hq(6)], wqk[:, hq(6)])    # j6 (K2)
            nc.sync.dma_start(wqkp[:, hq(7)], wqk[:, hq(7)])      # j7 (K3)
            nc.scalar.dma_start(wqkp[:, hq(2)], wqk[:, hq(2)])    # j2 (Q2)
            nc.gpsimd.dma_start(wqkp[:, hq(3)], wqk[:, hq(3)])    # j3 (Q3)
            nc.sync.dma_start(xtp[:, xq(2)], xT[:, xq(2)])        # c1 cb01
            nc.scalar.dma_start(xtp[:, xq(3)], xT[:, xq(3)])      # c1 cb23
            nc.gpsimd.dma_start(wvp[:], wv[:, :])
            nc.scalar.dma_start(wpp[:], wpTp[:, :])
            nc.sync.dma_start(bpr[:], bprow[:, :])

            def w_sb(jb, cb):      # [128 c_in, 128 j]  (Q/K weight lhsT)
                return wqkp[:, jb * 512 + cb * P:jb * 512 + (cb + 1) * P]

            def xt_ch(ch, cb):     # [128 c_in, 512 n]  (x^T rhs chunk)
                return xtp[:, ch * 2048 + cb * 512:ch * 2048 + (cb + 1) * 512]

            def xt_m(mb, cb):      # [128 c_in, 128 m]  (x^T lhsT m-block)
                base = (mb // 4) * 2048 + cb * 512 + (mb % 4) * P
                return xtp[:, base:base + P]

            def wv_sb(cb):         # [128 c_in, 512 c'] (V weight rhs)
                return wvp[:, cb * C:(cb + 1) * C]

            def wp_sb(ib):         # [128 (ib,d), 512 c_out] (proj rhs)
                return wpp[:, ib * C:(ib + 1) * C]

            # ---- PE p-state warmup while the first loads are in flight ----
            for w in range(N_WARMUP_MM):
                pw = pspool.tile([P, N], FP32, tag="ps", name=f"warm{w}")
                nc.tensor.matmul(pw[:, 0:512], ones_m[:], warm[:],
                                 start=True, stop=True)

            # ---- QKV projections, split per chunk ----
            qhat = [[qkvpool.tile([P, N], BF16, tag=f"qhat{i}_{h}",
                                  name=f"qhat{i}_{h}")
                     for h in range(H)] for i in range(H)]
            kt = [qkvpool.tile([P, N], BF16, tag=f"kt{h}", name=f"kt{h}")
                  for h in range(H)]
            v_w = [qkvpool.tile([P, N], BF16, tag=f"v{p}", name=f"v{p}")
                   for p in range(NB // 2)]

            def qk_pair(jbs, ch):
                # two jb half-projections side by side in one 2-bank tile,
                # then their PSUM->SBUF epilogues
                ps = pspool.tile([P, N], FP32, tag="ps",
                                 name=f"qk{jbs[0]}_{ch}")
                for sl, jb in enumerate(jbs):
                    for cb in range(CB):
                        nc.tensor.matmul(
                            ps[:, sl * 512:(sl + 1) * 512],
                            w_sb(jb, cb), xt_ch(ch, cb),
                            start=(cb == 0), stop=(cb == CB - 1),
                        )
                csl = slice(ch * 512, (ch + 1) * 512)
                for sl, jb in enumerate(jbs):
                    psl = slice(sl * 512, (sl + 1) * 512)
                    if jb < H:
                        h = jb
                        qb = qkvpool.tile([P, 512], BF16, tag=f"qb{h}_{ch}",
                                          name=f"qb{h}_{ch}")
                        nc.scalar.copy(qb[:], ps[:, psl])
                        for i in range(H):
                            sc = qsc[:, i * H + h:i * H + h + 1]
                            nc.vector.tensor_scalar_mul(
                                qhat[i][h][:, csl], qb[:], sc)
                    else:
                        nc.scalar.copy(kt[jb - H][:, csl], ps[:, psl])

            def v_group(p):
                # V: wide pair of m-blocks, one DVE cast per pair
                ps = pspool.tile([P, N], FP32, tag="ps", name=f"v_ps{p}")
                for half in range(2):
                    for cb in range(CB):
                        nc.tensor.matmul(
                            ps[:, half * 512:(half + 1) * 512],
                            xt_m(2 * p + half, cb), wv_sb(cb),
                            start=(cb == 0), stop=(cb == CB - 1),
                        )
                nc.vector.tensor_copy(v_w[p][:], ps[:])

            # ---- scores -> exp -> attnv/rowsum ----
            onorm = [opool.tile([P, N], BF16, tag=f"onorm{i}", name=f"onorm{i}")
                     for i in range(H)]

            def score_group(ch, mp, ip, es, ee):
                nsl = slice(ch * 512, (ch + 1) * 512)
                ii = (2 * ip, 2 * ip + 1)
                wides = {}
                for i in ii:
                    wides[i] = pspool.tile([P, N], FP32, tag="ps",
                                           name=f"s{ch}_{i}_{mp}")
                # h-outer / half / i-inner: each kt stationary is consumed
                # by 2 matmuls back to back (i pair)
                for h in range(H):
                    for half in range(2):
                        mb = 2 * mp + half
                        klhs = kt[h][:, mb * P:(mb + 1) * P]
                        for i in ii:
                            nc.tensor.matmul(
                                wides[i][:, half * 512:(half + 1) * 512],
                                klhs, qhat[i][h][:, nsl],
                                start=(h == 0), stop=(h == H - 1),
                            )
                for i in ii:
                    e = espool.tile([P, N], BF16, tag="es",
                                    name=f"es{ch}_{i}_{mp}")
                    nc.scalar.activation(e[:], wides[i][:], AF.Exp)
                    es[i][mp] = e
                    # DVE pre-reduction (same-engine, no cross-engine sems)
                    if mp == 1:
                        ee[i][0] = sumpool.tile([P, N], BF16, tag="ee",
                                                name=f"ee{ch}_{i}_0")
                        nc.vector.tensor_add(ee[i][0][:], es[i][0][:],
                                             es[i][1][:])
                    elif mp == 3:
                        ee[i][1] = sumpool.tile([P, N], BF16, tag="ee",
                                                name=f"ee{ch}_{i}_1")
                        nc.vector.tensor_add(ee[i][1][:], es[i][2][:],
                                             es[i][3][:])

            def tail_head(ch, i, es, ee):
                # attnv first (hides the DVE rowsum chain), then
                # rowsum -> recip -> normalize
                nsl = slice(ch * 512, (ch + 1) * 512)
                ew = sumpool.tile([P, N], BF16, tag="ew", name=f"ew{ch}_{i}")
                nc.vector.tensor_add(ew[:], ee[i][0][:], ee[i][1][:])
                etot = sumpool.tile([P, 512], BF16, tag="etot",
                                    name=f"etot{ch}_{i}")
                nc.vector.tensor_add(etot[:], ew[:, 0:512], ew[:, 512:N])
                rso = pspool.tile([P, N], FP32, tag="ps", name=f"rso{ch}_{i}")
                for mb in range(NB):
                    vl = v_w[mb // 2][:, (mb % 2) * 512 + i * P:
                                      (mb % 2) * 512 + (i + 1) * P]
                    nc.tensor.matmul(
                        rso[:, 512:N], vl,
                        es[i][mb // 2][:, (mb % 2) * 512:(mb % 2 + 1) * 512],
                        start=(mb == 0), stop=(mb == NB - 1),
                    )
                nc.tensor.matmul(rso[:, 0:512], ones_m[:], etot[:],
                                 start=True, stop=True)
                rec = outpool.tile([P, 512], FP32, tag="rec",
                                   name=f"rec{ch}_{i}")
                nc.vector.reciprocal_approx_fast(rec[:], rso[:, 0:512])
                nc.vector.tensor_mul(onorm[i][:, nsl], rso[:, 512:N], rec[:])

            def emit_proj_chunk(ch):
                # out rows [ch*512, ch*512+512); ib-ascending accumulation
                # so only the last matmul waits on the last head's norm;
                # output DMAs alternate between the two HWDGE queues
                for pp in range(2):
                    nbs = [ch * 4 + 2 * pp, ch * 4 + 2 * pp + 1]
                    ps = pspool.tile([P, N], FP32, tag="ps",
                                     name=f"p_ps{ch}_{pp}")
                    for half in range(2):
                        nb = nbs[half]
                        for ib in range(H):
                            nc.tensor.matmul(
                                ps[:, half * 512:(half + 1) * 512],
                                onorm[ib][:, nb * P:(nb + 1) * P],
                                wp_sb(ib),
                                start=(ib == 0), stop=(ib == H - 1),
                            )
                    osb = outpool.tile([P, N], BF16, tag="osb",
                                       name=f"osb{ch}_{pp}")
                    for half in range(2):
                        nb = nbs[half]
                        sl = slice(half * 512, (half + 1) * 512)
                        nc.vector.tensor_add(osb[:, sl], ps[:, sl], bpr[:])
                        eng = nc.sync if (half == 0) else nc.scalar
                        eng.dma_start(out[nb * P:(nb + 1) * P, :],
                                      osb[:, sl])

            def emit_proj_last(ch):
                # ib0-2 for both PSUM halves first; the final per-pp ib3
                # matmuls (the only ones gated on head 3's normalize) run
                # last, so the closing dependence chain is two matmuls,
                # one DVE add and one DMA per quarter.
                pss = []
                for pp in range(2):
                    ps = pspool.tile([P, N], FP32, tag="ps",
                                     name=f"p_ps{ch}_{pp}")
                    pss.append(ps)
                    for half in range(2):
                        nb = ch * 4 + 2 * pp + half
                        for ib in range(H - 1):
                            nc.tensor.matmul(
                                ps[:, half * 512:(half + 1) * 512],
                                onorm[ib][:, nb * P:(nb + 1) * P],
                                wp_sb(ib),
                                start=(ib == 0), stop=False,
                            )
                for pp in range(2):
                    ps = pss[pp]
                    osb = outpool.tile([P, N], BF16, tag="osb",
                                       name=f"osb{ch}_{pp}")
                    for half in range(2):
                        nb = ch * 4 + 2 * pp + half
                        sl = slice(half * 512, (half + 1) * 512)
                        nc.tensor.matmul(
                            ps[:, sl],
                            onorm[H - 1][:, nb * P:(nb + 1) * P],
                            wp_sb(H - 1),
                            start=False, stop=True,
                        )
                        nc.vector.tensor_add(osb[:, sl], ps[:, sl], bpr[:])
                        if pp == 0:
                            # third quarter rides the idle gpsimd queue so
                            # sync/scalar each issue only one final trigger
                            # (the trigger serialization gates drain entry)
                            eng = nc.gpsimd
                        else:
                            eng = nc.sync if (half == 0) else nc.scalar
                        eng.dma_start(out[nb * P:(nb + 1) * P, :],
                                      osb[:, sl])

            # ---- emission schedule ----
            # prologue: K(ch0) + Q(ch0) so ch0 scores start early
            qk_pair((H + 0, H + 1), 0)
            qk_pair((0, 1), 0)
            qk_pair((H + 2, H + 3), 0)
            qk_pair((2, 3), 0)

            es0 = [[None] * (NB // 2) for _ in range(H)]
            ee0 = [[None, None] for _ in range(H)]
            es1 = [[None] * (NB // 2) for _ in range(H)]
            ee1 = [[None, None] for _ in range(H)]

            # ch0 scores mp0; K(ch1) under them (kt m>=512 needed at mp2)
            score_group(0, 0, 0, es0, ee0)
            qk_pair((H + 0, H + 1), 1)
            score_group(0, 0, 1, es0, ee0)
            qk_pair((H + 2, H + 3), 1)
            # mp1; Q(ch1) under them (qhat ch1 needed at score ch1)
            score_group(0, 1, 0, es0, ee0)
            qk_pair((0, 1), 1)
            score_group(0, 1, 1, es0, ee0)
            qk_pair((2, 3), 1)
            # mp2; V under them (v_w needed at ch0 tail)
            score_group(0, 2, 0, es0, ee0)
            v_group(0); v_group(1)
            score_group(0, 2, 1, es0, ee0)
            v_group(2); v_group(3)
            score_group(0, 3, 0, es0, ee0)
            score_group(0, 3, 1, es0, ee0)

            # one ch1 group first so ch0's last exp latency is hidden
            score_group(1, 0, 0, es1, ee1)
            for i in range(H):
                tail_head(0, i, es0, ee0)
            emit_proj_chunk(0)

            # last chunk: pair-outer; pair-0 tail hides under pair-1 groups
            score_group(1, 1, 0, es1, ee1)
            score_group(1, 2, 0, es1, ee1)
            score_group(1, 3, 0, es1, ee1)
            score_group(1, 0, 1, es1, ee1)
            tail_head(1, 0, es1, ee1)
            tail_head(1, 1, es1, ee1)
            score_group(1, 1, 1, es1, ee1)
            score_group(1, 2, 1, es1, ee1)
            score_group(1, 3, 1, es1, ee1)
            tail_head(1, 2, es1, ee1)
            tail_head(1, 3, es1, ee1)
            emit_proj_last(1)

    nc.compile()
    return nc


def _pack_cb(a):
    """[C, W] -> [128, CB*W]: partition-block cb at free offset cb*W."""
    Crows, W = a.shape
    return np.ascontiguousarray(
        a.reshape(Crows // P, P, W).transpose(1, 0, 2).reshape(P, -1)
    )


def _pack_xT(xb):
    """x[b].T [C, N] -> [128, NCH*(CB*512)] ch-major, cb-minor."""
    a = xb.reshape(CB, P, NCH, 512).transpose(1, 2, 0, 3)
    return np.ascontiguousarray(a.reshape(P, -1))


def _pack_wqk(wqkT):
    """w_qkv.T[:, :2C] [C, 2C] -> [128, 8*(CB*128)] jb-major, cb-minor."""
    a = wqkT.reshape(CB, P, 2 * H, P).transpose(1, 2, 0, 3)
    return np.ascontiguousarray(a.reshape(P, -1))


def make_in_maps(x, w_qkv, w_proj, b_proj, w_main, w_rest):
    M = _mix_matrix_np(np.asarray(w_main), np.asarray(w_rest))
    bf = ml_dtypes.bfloat16
    wqkvT = np.ascontiguousarray(np.asarray(w_qkv, np.float32).T).astype(bf)
    wpT = np.ascontiguousarray(np.asarray(w_proj, np.float32).T).astype(bf)
    bprow = np.broadcast_to(
        np.asarray(b_proj, np.float32).reshape(1, C), (P, C)
    ).astype(bf)
    qs = np.empty((P, H * H), np.float32)
    for i in range(H):
        for h in range(H):
            qs[:, i * H + h] = np.float32(M[i, h] * SCALE)
    x = np.asarray(x, np.float32)
    wqk_p = _pack_wqk(wqkvT[:, 0:2 * C])
    wv_p = _pack_cb(wqkvT[:, 2 * C:3 * C])
    wp_p = _pack_cb(wpT)
    in_maps = []
    for b in range(B):
        in_maps.append({
            "xT": _pack_xT(np.ascontiguousarray(x[b].T).astype(bf)),
            "wqk": wqk_p,
            "wv": wv_p,
            "wpTp": wp_p,
            "bprow": bprow,
            "qscales": qs,
        })
    return in_maps


_NC_CACHE = {}


def get_graph():
    if "nc" not in _NC_CACHE:
        _NC_CACHE["nc"] = build_graph()
    return _NC_CACHE["nc"]


def kernel(x, w_qkv, w_proj, b_proj, w_main, w_rest, _trace=False, _trace_kwargs=None):
    nc = get_graph()
    in_maps = make_in_maps(x, w_qkv, w_proj, b_proj, w_main, w_rest)
    kw = {}
    if _trace:
        kw = {"trace": True}
        if _trace_kwargs:
            kw.update(_trace_kwargs)
    res = run_bass_kernel_spmd(nc, in_maps, core_ids=list(range(NCORES)), **kw)
    outb = np.stack([res.results[i]["out"] for i in range(NCORES)], axis=0)
    if _trace:
        return outb.astype(np.float32), res
    return outb.astype(np.float32)


# revision 7
# speedup vs baseline: 1.0089x; 1.0089x over previous
"""Trainium2 Bass kernel for mixed-head attention (CIM attention).

Reference computation (per batch element b):
    qkv  = x @ w_qkv.T                                  [N, 3C]
    q,k,v split into H=4 heads of HD=128
    S_h  = (q_h @ k_h.T) * SCALE                        [N, N] per head
    S'_i = sum_h M[i,h] * S_h        (CIM head mix)
    A_i  = softmax(S'_i, axis=-1)
    O_i  = A_i @ v_i
    out  = concat_i(O_i) @ w_proj.T + b_proj

Distribution: data-parallel over B=8, one batch element per NeuronCore.
No collectives needed; host shards/gathers.

Single-core algorithm (all matmuls bf16 with fp32 PSUM accumulation):
  - Host ships x^T, w_qkv^T, w_proj^T pre-transposed, pre-cast to bf16 and
    pre-packed into flat "SBUF image" layouts; inputs stream in first-use
    order as 128 KB pieces over the three DMA queues; gpsimd runs the
    memsets first so the ones tile is ready for a short burst of
    throwaway ones*zeros matmuls that warms the PE p-state while the
    first operands are in flight.
  - The QK projection is split per 512-wide chunk, ordered Q(ch0),
    K(ch0), Q(ch0 hi-heads), K(ch0 hi-heads), so ch0 score groups start
    after ~1 MB of input instead of 2 MB; the first two score groups are
    additionally split into low/high-head halves so the PE never waits
    for the tail of the K epilogues.  K(ch1), Q(ch1) and V are emitted
    between ch0 score groups and hide under them.
  - The CIM mix is folded into Q: Qhat_i[(h,d), n] = M[i,h]*SCALE*Q_h[d, n],
    applied by DVE tensor_scalar directly from the projection PSUM (no
    intermediate copy).  K is cast PSUM->SBUF on ACT.  The score matmul
    contracts over all 512 (h,d) pairs:
    S'_i^T[m, n] = sum_{(h,d)} K[(h,d), m] * Qhat_i[(h,d), n].
  - Scores live in [m_part, n_free] ("S^T") layout so exp is elementwise and
    attn@v needs no transpose: O_i^T[d, n] = sum_m V[m, d] * expS_i^T[m, n].
  - Score tiles are computed in [128, 1024] 2-bank PSUM pairs (two m-blocks
    side by side), with an i-pair inner loop so each K stationary load is
    reused by 4 consecutive matmuls; one wide exp per pair halves the ACT
    instruction count (and the semaphore count, which sets the length of
    the framework's teardown ladder).
  - Softmax normalization is deferred past attn@v.  Denominators come from
    a DVE add chain over the wide exp tiles feeding one ones[128,128]
    stationary matmul whose M=128 output broadcasts the rowsum to all
    partitions; 1/rowsum via reciprocal_approx_fast, applied to O^T with
    tensor_mul.  Per chunk the tail runs attnv before each head's rowsum
    so the DVE chain is off the PE critical path, and every tail is
    emitted after one score group of the following phase so the exp
    latency of its last tile is hidden.
  - The last chunk runs its score groups head-pair-outer: the pair-0 tail
    (attnv + normalize for heads 0,1) hides under pair-1's score groups,
    so only one head pair's normalize chain sits on the critical path at
    the very end.
  - proj: the bias rides as a leading ones[128,128] @ (b/128) matmul into
    each PSUM accumulation (host ships b_proj/128), so the epilogue is a
    pure dtype cast — on DVE for chunk 0 and split ACT/DVE per quarter
    for the final chunk so the four closing casts run on two engines.
    Chunk 0's proj is emitted after the first score group of chunk 1;
    the final chunk's proj runs ib0-2 for both PSUM halves first and ib3
    last, so only the last two matmuls wait on head 3.  Output DMAs ride
    sync/scalar/vector only — the gpsimd SWDGE queue drains slowest at
    teardown, so it gets no late work.
"""

import os
import sys

for _p in ("/opt/trn_rl_repo",):
    if os.path.isdir(_p) and _p not in sys.path:
        sys.path.insert(0, _p)

import numpy as np
import ml_dtypes

import concourse.bass as bass
import concourse.tile as tile
from concourse import bacc, mybir
from concourse.bass_utils import run_bass_kernel_spmd

B, N, C, H = 8, 1024, 512, 4
HD = C // H          # 128
SCALE = HD ** -0.5
NCORES = 8
P = 128              # partitions
NCH = N // 512       # 512-wide free-dim chunks per N
NB = N // P          # 128-row blocks per N
CB = C // P          # 128-row blocks per C

BF16 = mybir.dt.bfloat16
FP32 = mybir.dt.float32
AF = mybir.ActivationFunctionType

N_WARMUP_MM = 8      # ones*zeros p-state warmers while first DMAs land


def _mix_matrix_np(w_main: np.ndarray, w_rest: np.ndarray) -> np.ndarray:
    rows = np.repeat(np.arange(H), H - 1)
    cols = np.array([[j for j in range(H) if j != i] for i in range(H)]).ravel()
    M = np.zeros((H, H), dtype=np.float64)
    M[rows, cols] = w_rest.astype(np.float64).ravel()
    M += np.diag(w_main.astype(np.float64))
    return M


def build_graph():
    nc = bacc.Bacc(
        "TRN2",
        target_bir_lowering=False,
        debug=False,
        num_devices=NCORES,
    )

    # xT: ch-major [128, NCH*(CB*512)]; block (ch, cb) at ch*2048 + cb*512
    xT = nc.dram_tensor("xT", [P, NCH * CB * 512], BF16, kind="ExternalInput").ap()
    # wqk: jb-major [128, 8*(CB*128)]; block (jb, cb) at jb*512 + cb*128
    wqk = nc.dram_tensor("wqk", [P, 2 * H * CB * P], BF16, kind="ExternalInput").ap()
    # wv / wpT: cb-major [128, CB*512]
    wv = nc.dram_tensor("wv", [P, CB * C], BF16, kind="ExternalInput").ap()
    wpTp = nc.dram_tensor("wpTp", [P, CB * C], BF16, kind="ExternalInput").ap()
    # b_proj / 128, broadcast to all partitions (see bias matmul)
    bprow = nc.dram_tensor("bprow", [P, C], BF16, kind="ExternalInput").ap()
    qscales = nc.dram_tensor("qscales", [P, H * H], FP32, kind="ExternalInput").ap()
    # bf16 output (upcast host-side): halves the output-DMA tail; the
    # ~0.1% quantization is well inside the error budget
    out = nc.dram_tensor("out", [N, C], BF16, kind="ExternalOutput").ap()

    with tile.TileContext(nc, pool_alloc_mode="queue") as tc:
        with (
            tc.tile_pool(name="const", bufs=1) as cpool,
            tc.tile_pool(name="wts", bufs=1) as wpool,
            tc.tile_pool(name="qkv", bufs=1) as qkvpool,
            tc.tile_pool(name="es", bufs=16) as espool,
            tc.tile_pool(name="esum", bufs=8) as sumpool,
            tc.tile_pool(name="onorm", bufs=1) as opool,
            tc.tile_pool(name="outsb", bufs=3) as outpool,
            tc.tile_pool(name="ps", bufs=4, space="PSUM") as pspool,
        ):
            # gpsimd: memsets first, so ones_m is ready early for warmup
            ones_m = cpool.tile([P, P], BF16, tag="ones_m")
            nc.gpsimd.memset(ones_m[:], 1.0)
            warm = cpool.tile([P, 512], BF16, tag="warm")
            nc.gpsimd.memset(warm[:], 0.0)

            # ---- staged input DMA, three queues, first-use order ----
            wqkp = wpool.tile([P, 2 * H * CB * P], BF16, tag="wqkp", name="wqkp")
            xtp = wpool.tile([P, NCH * CB * 512], BF16, tag="xtp", name="xtp")
            wvp = wpool.tile([P, CB * C], BF16, tag="wvp", name="wvp")
            wpp = wpool.tile([P, CB * C], BF16, tag="wpp", name="wpp")
            qsc = cpool.tile([P, H * H], FP32, tag="qsc")
            bpr = cpool.tile([P, C], BF16, tag="bpr")

            def xq(p):
                return slice(p * 1024, (p + 1) * 1024)

            def hq(p):
                return slice(p * 512, (p + 1) * 512)

            # First-use order per queue: x ch0 + Q weights (j0,j1) lead,
            # then K (j4,j5), Q hi (j2,j3), K hi (j6,j7); x ch1 and the
            # late tensors (wv, wpT, bias) trail.
            nc.scalar.dma_start(qsc[:], qscales[:, :])
            nc.sync.dma_start(xtp[:, hq(0)], xT[:, hq(0)])        # c0 cb0
            nc.scalar.dma_start(xtp[:, hq(1)], xT[:, hq(1)])      # c0 cb1
            nc.gpsimd.dma_start(xtp[:, hq(2)], xT[:, hq(2)])      # c0 cb2
            nc.sync.dma_start(wqkp[:, hq(0)], wqk[:, hq(0)])      # j0 (Q0)
            nc.scalar.dma_start(wqkp[:, hq(1)], wqk[:, hq(1)])    # j1 (Q1)
            nc.gpsimd.dma_start(xtp[:, hq(3)], xT[:, hq(3)])      # c0 cb3
            nc.sync.dma_start(wqkp[:, hq(4)], wqk[:, hq(4)])      # j4 (K0)
            nc.scalar.dma_start(wqkp[:, hq(5)], wqk[:, hq(5)])    # j5 (K1)
            nc.gpsimd.dma_start(wqkp[:, hq(6)], wqk[:, hq(6)])    # j6 (K2)
            nc.sync.dma_start(wqkp[:, hq(2)], wqk[:, hq(2)])      # j2 (Q2)
            nc.scalar.dma_start(wqkp[:, hq(3)], wqk[:, hq(3)])    # j3 (Q3)
            nc.gpsimd.dma_start(wqkp[:, hq(7)], wqk[:, hq(7)])    # j7 (K3)
            nc.sync.dma_start(xtp[:, xq(2)], xT[:, xq(2)])        # c1 cb01
            nc.scalar.dma_start(xtp[:, xq(3)], xT[:, xq(3)])      # c1 cb23
            nc.gpsimd.dma_start(wvp[:], wv[:, :])
            nc.scalar.dma_start(wpp[:], wpTp[:, :])
            nc.sync.dma_start(bpr[:], bprow[:, :])

            def w_sb(jb, cb):      # [128 c_in, 128 j]  (Q/K weight lhsT)
                return wqkp[:, jb * 512 + cb * P:jb * 512 + (cb + 1) * P]

            def xt_ch(ch, cb):     # [128 c_in, 512 n]  (x^T rhs chunk)
                return xtp[:, ch * 2048 + cb * 512:ch * 2048 + (cb + 1) * 512]

            def xt_m(mb, cb):      # [128 c_in, 128 m]  (x^T lhsT m-block)
                base = (mb // 4) * 2048 + cb * 512 + (mb % 4) * P
                return xtp[:, base:base + P]

            def wv_sb(cb):         # [128 c_in, 512 c'] (V weight rhs)
                return wvp[:, cb * C:(cb + 1) * C]

            def wp_sb(ib):         # [128 (ib,d), 512 c_out] (proj rhs)
                return wpp[:, ib * C:(ib + 1) * C]

            # ---- PE p-state warmup while the first loads are in flight ----
            for w in range(N_WARMUP_MM):
                pw = pspool.tile([P, N], FP32, tag="ps", name=f"warm{w}")
                nc.tensor.matmul(pw[:, 0:512], ones_m[:], warm[:],
                                 start=True, stop=True)

            # ---- QKV projections, split per chunk ----
            qhat = [[qkvpool.tile([P, N], BF16, tag=f"qhat{i}_{h}",
                                  name=f"qhat{i}_{h}")
                     for h in range(H)] for i in range(H)]
            kt = [qkvpool.tile([P, N], BF16, tag=f"kt{h}", name=f"kt{h}")
                  for h in range(H)]
            v_w = [qkvpool.tile([P, N], BF16, tag=f"v{p}", name=f"v{p}")
                   for p in range(NB // 2)]

            def qk_pair(jbs, ch):
                # two jb half-projections side by side in one 2-bank tile;
                # Q epilogues run on DVE straight from PSUM, K on ACT
                ps = pspool.tile([P, N], FP32, tag="ps",
                                 name=f"qk{jbs[0]}_{ch}")
                for sl, jb in enumerate(jbs):
                    for cb in range(CB):
                        nc.tensor.matmul(
                            ps[:, sl * 512:(sl + 1) * 512],
                            w_sb(jb, cb), xt_ch(ch, cb),
                            start=(cb == 0), stop=(cb == CB - 1),
                        )
                    csl = slice(ch * 512, (ch + 1) * 512)
                    psl = slice(sl * 512, (sl + 1) * 512)
                    if jb < H:
                        h = jb
                        for i in range(H):
                            sc = qsc[:, i * H + h:i * H + h + 1]
                            nc.vector.tensor_scalar_mul(
                                qhat[i][h][:, csl], ps[:, psl], sc)
                    else:
                        nc.scalar.copy(kt[jb - H][:, csl], ps[:, psl])

            def v_group(p):
                # V: wide pair of m-blocks, one DVE cast per pair
                ps = pspool.tile([P, N], FP32, tag="ps", name=f"v_ps{p}")
                for half in range(2):
                    for cb in range(CB):
                        nc.tensor.matmul(
                            ps[:, half * 512:(half + 1) * 512],
                            xt_m(2 * p + half, cb), wv_sb(cb),
                            start=(cb == 0), stop=(cb == CB - 1),
                        )
                nc.vector.tensor_copy(v_w[p][:], ps[:])

            # ---- scores -> exp -> attnv/rowsum ----
            onorm = [opool.tile([P, N], BF16, tag=f"onorm{i}", name=f"onorm{i}")
                     for i in range(H)]

            def score_group(ch, mp, ip, es, ee, hs=(0, 1, 2, 3), wides=None,
                            finish=True):
                nsl = slice(ch * 512, (ch + 1) * 512)
                ii = (2 * ip, 2 * ip + 1)
                if wides is None:
                    wides = {}
                    for i in ii:
                        wides[i] = pspool.tile([P, N], FP32, tag="ps",
                                               name=f"s{ch}_{i}_{mp}")
                # h-outer / half / i-inner: each kt stationary is consumed
                # by 2 matmuls back to back (i pair)
                for h in hs:
                    for half in range(2):
                        mb = 2 * mp + half
                        klhs = kt[h][:, mb * P:(mb + 1) * P]
                        for i in ii:
                            nc.tensor.matmul(
                                wides[i][:, half * 512:(half + 1) * 512],
                                klhs, qhat[i][h][:, nsl],
                                start=(h == 0), stop=(h == H - 1),
                            )
                if not finish:
                    return wides
                for i in ii:
                    e = espool.tile([P, N], BF16, tag="es",
                                    name=f"es{ch}_{i}_{mp}")
                    nc.scalar.activation(e[:], wides[i][:], AF.Exp)
                    es[i][mp] = e
                    # DVE pre-reduction (same-engine, no cross-engine sems)
                    if mp == 1:
                        ee[i][0] = sumpool.tile([P, N], BF16, tag="ee",
                                                name=f"ee{ch}_{i}_0")
                        nc.vector.tensor_add(ee[i][0][:], es[i][0][:],
                                             es[i][1][:])
                    elif mp == 3:
                        ee[i][1] = sumpool.tile([P, N], BF16, tag="ee",
                                                name=f"ee{ch}_{i}_1")
                        nc.vector.tensor_add(ee[i][1][:], es[i][2][:],
                                             es[i][3][:])
                return wides

            def tail_head(ch, i, es, ee):
                # attnv first (hides the DVE rowsum chain), then
                # rowsum -> recip -> normalize
                nsl = slice(ch * 512, (ch + 1) * 512)
                ew = sumpool.tile([P, N], BF16, tag="ew", name=f"ew{ch}_{i}")
                nc.vector.tensor_add(ew[:], ee[i][0][:], ee[i][1][:])
                etot = sumpool.tile([P, 512], BF16, tag="etot",
                                    name=f"etot{ch}_{i}")
                nc.vector.tensor_add(etot[:], ew[:, 0:512], ew[:, 512:N])
                rso = pspool.tile([P, N], FP32, tag="ps", name=f"rso{ch}_{i}")
                for mb in range(NB):
                    vl = v_w[mb // 2][:, (mb % 2) * 512 + i * P:
                                      (mb % 2) * 512 + (i + 1) * P]
                    nc.tensor.matmul(
                        rso[:, 512:N], vl,
                        es[i][mb // 2][:, (mb % 2) * 512:(mb % 2 + 1) * 512],
                        start=(mb == 0), stop=(mb == NB - 1),
                    )
                nc.tensor.matmul(rso[:, 0:512], ones_m[:], etot[:],
                                 start=True, stop=True)
                rec = outpool.tile([P, 512], FP32, tag="rec",
                                   name=f"rec{ch}_{i}")
                nc.vector.reciprocal_approx_fast(rec[:], rso[:, 0:512])
                nc.vector.tensor_mul(onorm[i][:, nsl], rso[:, 512:N], rec[:])

            def proj_psum(ps, half, nb, ibs, start, stop):
                hsl = slice(half * 512, (half + 1) * 512)
                if start:
                    # bias as a leading matmul: ones[128,128] @ (b/128)
                    nc.tensor.matmul(ps[:, hsl], ones_m[:], bpr[:],
                                     start=True, stop=False)
                for ib in ibs:
                    nc.tensor.matmul(
                        ps[:, hsl],
                        onorm[ib][:, nb * P:(nb + 1) * P],
                        wp_sb(ib),
                        start=False, stop=(stop and ib == ibs[-1]),
                    )

            def emit_proj_chunk(ch):
                # out rows [ch*512, ch*512+512); ib-ascending accumulation
                # so only the last matmul waits on the last head's norm
                for pp in range(2):
                    nbs = [ch * 4 + 2 * pp, ch * 4 + 2 * pp + 1]
                    ps = pspool.tile([P, N], FP32, tag="ps",
                                     name=f"p_ps{ch}_{pp}")
                    for half in range(2):
                        proj_psum(ps, half, nbs[half], list(range(H)),
                                  True, True)
                    osb = outpool.tile([P, N], BF16, tag="osb",
                                       name=f"osb{ch}_{pp}")
                    nc.vector.tensor_copy(osb[:], ps[:])
                    for half in range(2):
                        nb = nbs[half]
                        sl = slice(half * 512, (half + 1) * 512)
                        eng = nc.sync if (half == 0) else nc.scalar
                        eng.dma_start(out[nb * P:(nb + 1) * P, :],
                                      osb[:, sl])

            def emit_proj_last(ch):
                # ib0-2 for both PSUM halves first; the final per-pp ib3
                # matmuls (the only ones gated on head 3's normalize) run
                # last; per-quarter casts split across ACT and DVE; DMA
                # triggers on sync/scalar/vector (gpsimd drains slowest)
                pss = []
                for pp in range(2):
                    ps = pspool.tile([P, N], FP32, tag="ps",
                                     name=f"p_ps{ch}_{pp}")
                    pss.append(ps)
                    for half in range(2):
                        nb = ch * 4 + 2 * pp + half
                        proj_psum(ps, half, nb, [0, 1, 2], True, False)
                engs = [("v", nc.sync), ("a", nc.scalar),
                        ("v", nc.sync), ("a", nc.scalar)]
                q = 0
                for pp in range(2):
                    ps = pss[pp]
                    osb = outpool.tile([P, N], BF16, tag="osb",
                                       name=f"osb{ch}_{pp}")
                    for half in range(2):
                        nb = ch * 4 + 2 * pp + half
                        sl = slice(half * 512, (half + 1) * 512)
                        proj_psum(ps, half, nb, [3], False, True)
                        ceng, deng = engs[q]
                        q += 1
                        if ceng == "a":
                            nc.scalar.copy(osb[:, sl], ps[:, sl])
                        else:
                            nc.vector.tensor_copy(osb[:, sl], ps[:, sl])
                        deng.dma_start(out[nb * P:(nb + 1) * P, :],
                                       osb[:, sl])

            # ---- emission schedule ----
            # prologue: Q(ch0) then K(ch0), low heads first
            qk_pair((0, 1), 0)
            qk_pair((H + 0, H + 1), 0)
            qk_pair((2, 3), 0)
            qk_pair((H + 2, H + 3), 0)

            es0 = [[None] * (NB // 2) for _ in range(H)]
            ee0 = [[None, None] for _ in range(H)]
            es1 = [[None] * (NB // 2) for _ in range(H)]
            ee1 = [[None, None] for _ in range(H)]

            # first two ch0 score groups split by head half so the PE can
            # run on heads 0,1 while the head-2,3 epilogues finish
            w0 = score_group(0, 0, 0, es0, ee0, hs=(0, 1), finish=False)
            w1 = score_group(0, 0, 1, es0, ee0, hs=(0, 1), finish=False)
            score_group(0, 0, 0, es0, ee0, hs=(2, 3), wides=w0)
            score_group(0, 0, 1, es0, ee0, hs=(2, 3), wides=w1)
            # K(ch1) under mp1 (kt m>=512 needed at mp2)
            score_group(0, 1, 0, es0, ee0)
            qk_pair((H + 0, H + 1), 1)
            score_group(0, 1, 1, es0, ee0)
            qk_pair((H + 2, H + 3), 1)
            # Q(ch1) under mp2 (qhat ch1 needed at score ch1)
            score_group(0, 2, 0, es0, ee0)
            qk_pair((0, 1), 1)
            score_group(0, 2, 1, es0, ee0)
            qk_pair((2, 3), 1)
            # V under mp3 (v_w needed at ch0 tail)
            score_group(0, 3, 0, es0, ee0)
            v_group(0); v_group(1)
            score_group(0, 3, 1, es0, ee0)
            v_group(2); v_group(3)

            # one ch1 group first so ch0's last exp latency is hidden
            score_group(1, 0, 0, es1, ee1)
            for i in range(H):
                tail_head(0, i, es0, ee0)
            emit_proj_chunk(0)

            # last chunk: pair-outer; pair-0 tail hides under pair-1 groups
            score_group(1, 1, 0, es1, ee1)
            score_group(1, 2, 0, es1, ee1)
            score_group(1, 3, 0, es1, ee1)
            score_group(1, 0, 1, es1, ee1)
            tail_head(1, 0, es1, ee1)
            tail_head(1, 1, es1, ee1)
            score_group(1, 1, 1, es1, ee1)
            score_group(1, 2, 1, es1, ee1)
            score_group(1, 3, 1, es1, ee1)
            tail_head(1, 2, es1, ee1)
            tail_head(1, 3, es1, ee1)
            emit_proj_last(1)

    nc.compile()
    return nc


def _pack_cb(a):
    """[C, W] -> [128, CB*W]: partition-block cb at free offset cb*W."""
    Crows, W = a.shape
    return np.ascontiguousarray(
        a.reshape(Crows // P, P, W).transpose(1, 0, 2).reshape(P, -1)
    )


def _pack_xT(xb):
    """x[b].T [C, N] -> [128, NCH*(CB*512)] ch-major, cb-minor."""
    a = xb.reshape(CB, P, NCH, 512).transpose(1, 2, 0, 3)
    return np.ascontiguousarray(a.reshape(P, -1))


def _pack_wqk(wqkT):
    """w_qkv.T[:, :2C] [C, 2C] -> [128, 8*(CB*128)] jb-major, cb-minor."""
    a = wqkT.reshape(CB, P, 2 * H, P).transpose(1, 2, 0, 3)
    return np.ascontiguousarray(a.reshape(P, -1))


def make_in_maps(x, w_qkv, w_proj, b_proj, w_main, w_rest):
    M = _mix_matrix_np(np.asarray(w_main), np.asarray(w_rest))
    bf = ml_dtypes.bfloat16
    wqkvT = np.ascontiguousarray(np.asarray(w_qkv, np.float32).T).astype(bf)
    wpT = np.ascontiguousarray(np.asarray(w_proj, np.float32).T).astype(bf)
    # b/128: the kernel adds the bias via a ones[128,128] matmul
    bprow = np.broadcast_to(
        np.asarray(b_proj, np.float32).reshape(1, C) / P, (P, C)
    ).astype(bf)
    qs = np.empty((P, H * H), np.float32)
    for i in range(H):
        for h in range(H):
            qs[:, i * H + h] = np.float32(M[i, h] * SCALE)
    x = np.asarray(x, np.float32)
    wqk_p = _pack_wqk(wqkvT[:, 0:2 * C])
    wv_p = _pack_cb(wqkvT[:, 2 * C:3 * C])
    wp_p = _pack_cb(wpT)
    in_maps = []
    for b in range(B):
        in_maps.append({
            "xT": _pack_xT(np.ascontiguousarray(x[b].T).astype(bf)),
            "wqk": wqk_p,
            "wv": wv_p,
            "wpTp": wp_p,
            "bprow": bprow,
            "qscales": qs,
        })
    return in_maps


_NC_CACHE = {}


def get_graph():
    if "nc" not in _NC_CACHE:
        _NC_CACHE["nc"] = build_graph()
    return _NC_CACHE["nc"]


def kernel(x, w_qkv, w_proj, b_proj, w_main, w_rest, _trace=False, _trace_kwargs=None):
    nc = get_graph()
    in_maps = make_in_maps(x, w_qkv, w_proj, b_proj, w_main, w_rest)
    kw = {}
    if _trace:
        kw = {"trace": True}
        if _trace_kwargs:
            kw.update(_trace_kwargs)
    res = run_bass_kernel_spmd(nc, in_maps, core_ids=list(range(NCORES)), **kw)
    outb = np.stack([res.results[i]["out"] for i in range(NCORES)], axis=0)
    if _trace:
        return outb.astype(np.float32), res
    return outb.astype(np.float32)
